# revision 35
# baseline (speedup 1.0000x reference)
"""BitNet 4-layer MLP (8192x4096, ternary weights, int8-style activations)
on 8 Trainium2 NeuronCores.

Strategy: pure data-parallel over the 8192-token dim (1024 tokens/core, no
collectives). Activations live TRANSPOSED on chip ([feature, token]) so the
output of each layer's matmul (PSUM [out_feat, tok]) is directly the next
layer's moving operand — zero transposes on device. Weights are quantized
to ternary bf16 on the host (matmul over {-1,0,1} x integers <= 128 is
exact in bf16 with fp32 PSUM accumulation) and streamed per layer.

v2 vs baseline: LayerNorm stats no longer ride the PE per output tile.
The 32 h tiles of a half are accumulated on DVE (sum and sum-of-squares),
and a single ones-vector matmul per stat does the final 128-partition
reduction — 12 stats matmuls total instead of 768. Stats/broadcast
matmuls for a finished half are emitted a few tiles into the NEXT half's
main matmuls so the in-order PE queue never stalls on DVE row math.
Weight DMA is prefetched 3 tiles deep (kills the per-phase 2.4us gap),
gamma/beta load as one strided DMA per layer, and the quantize chain is
fused to 7 DVE ops per tile using the +/-1.5*2^23 magic-number round.

Each core processes its 1024 tokens as two 512-token halves pipelined
against each other: while half B's matmuls run on PE, half A's layernorm/
quantize chain runs on DVE, so PE never waits.
"""

import numpy as np

NUM_CORES = 8
N_TOK, D = 8192, 4096
NUM_LAYERS = 4
P = 128                      # SBUF partitions
KT = D // P                  # 32 k-tiles per contraction
NLOC = N_TOK // NUM_CORES    # 1024 tokens per core
HALF = 512                   # token half-chunk (one PSUM bank @ fp32)
MAGIC = 12582912.0           # 1.5 * 2**23: fp32 add/sub does RNE-to-integer

_prog_cache = {}


def _install_drain_patch():
    """walrus CoreV3 rejects instructions carrying >~2 embedded sem waits
    ("Too many sync wait commands"). Tile's exit drain waits on the whole
    vector clock; spread its waits across trailing sync-engine nops."""
    import concourse.tile as tile
    import concourse.mybir as mybir
    from concourse.tile import ScopedClock

    if getattr(tile.TileContext, "_drain_patch_installed", False):
        return

    def _patched(self, tick_clock, wait_clock):
        nc = self.nc
        drain_inst = nc.sync.drain()
        wait_clock.add_sem_waits(
            drain_inst.ins, ScopedClock({None: tick_clock.global_clock})
        )
        si = drain_inst.ins.sync_info
        waits = list(si.on_wait or []) if si is not None else []
        if len(waits) > 1:
            si.on_wait = waits[:1]
            for w in waits[1:]:
                nop = nc.sync.nop(nofuse=True)
                nsi = nop.ins.sync_info
                if nsi is None:
                    nop.ins.sync_info = mybir.SyncInfo(on_wait=[w], on_update=[])
                else:
                    nsi.on_wait = [w]
        nc.all_engine_barrier()
        assert self.sems is not None
        popped = nc._tile_sem_poison_stack.pop()
        assert popped is self._sem_poison
        nc.clear_and_free_semaphores(list(self.sems.allocated().values()))
        nc.all_engine_barrier()

    tile.TileContext._drain_and_barrier = _patched
    tile.TileContext._drain_patch_installed = True


def _split_excess_waits(nc, maxw=1):
    """walrus's per-instruction sync-wait encodings hold few waits; hoist
    excess waits onto same-engine nops spliced immediately before the
    overloaded instruction (adjacent on the same queue, so ordering
    semantics are unchanged)."""
    import copy
    import concourse.mybir as mybir

    ctr = [0]
    # a genuine InstNoOp prototype (left at stream end, harmless)
    proto = nc.sync.nop(nofuse=True)
    _NOP_PROTO = copy.deepcopy(proto.ins)
    _NOP_PROTO.sync_info = None

    def make_nop(proto_engine, waits):
        ctr[0] += 1
        nop = copy.deepcopy(_NOP_PROTO)
        nop.name = f"I-waitsplit-{ctr[0]}"
        nop.engine = proto_engine
        nop.sync_info = mybir.SyncInfo(on_wait=list(waits), on_update=[])
        return nop

    for bb in nc.m.functions[0].blocks:
        changed = False
        out = []
        for inst in bb.instructions:
            si = inst.sync_info
            waits = list(si.on_wait) if (si is not None and si.on_wait) else []
            if len(waits) > maxw and type(inst).__name__ != "InstISA":
                for i in range(0, len(waits) - maxw, maxw):
                    out.append(make_nop(inst.engine, waits[i:i + maxw]))
                si.on_wait = waits[len(waits) - maxw:]
                changed = True
            out.append(inst)
        if changed:
            bb.instructions = out
    return nc


def _trim_pe_sem_updates(nc):
    """Every MATMUL increments the PE clock semaphore at retire; the EVT_SEM
    write serializes against issue and costs a few ns per matmul. Consumers
    only ever need the *stop* matmul of an accumulation group (PSUM reads)
    or a later count (WAR tile reuse), so: drop the increment from non-stop
    matmuls and round every wait threshold on that semaphore up to the next
    retained increment. Rounding up releases waiters at-or-after the
    original point, which is always safe for acquire-style waits."""
    insts = []
    for bb in nc.m.functions[0].blocks:
        insts.extend(bb.instructions)

    pe_sem = None
    for inst in insts:
        if type(inst).__name__ == "InstMatmult":
            si = inst.sync_info
            if si and si.on_update:
                for u in si.on_update:
                    if u.update_mode == "sem-inc":
                        pe_sem = u.id
                        break
            if pe_sem is not None:
                break
    if pe_sem is None:
        return nc

    # clocked PE instructions in program order
    clocked = []
    for inst in insts:
        if str(inst.engine) != "EngineType.PE":
            continue
        si = inst.sync_info
        if si and si.on_update and any(
                u.id == pe_sem and u.update_mode == "sem-inc"
                for u in si.on_update):
            keep = (type(inst).__name__ != "InstMatmult"
                    or bool(inst.stop_tensor_calc))
            clocked.append((inst, keep))

    K = len(clocked)
    kept_prefix = [0] * (K + 1)
    for i, (inst, keep) in enumerate(clocked, start=1):
        kept_prefix[i] = kept_prefix[i - 1] + (1 if keep else 0)
    # for old threshold t: new threshold = kept count at the nearest kept
    # index >= t (if none, total kept)
    next_kept_at = [0] * (K + 1)
    nk = kept_prefix[K]
    for t in range(K, 0, -1):
        if clocked[t - 1][1]:
            nk = kept_prefix[t]
        next_kept_at[t] = nk

    # rewrite waits everywhere
    for inst in insts:
        si = inst.sync_info
        if si is None or not si.on_wait:
            continue
        for w in si.on_wait:
            if w.id == pe_sem and w.wait_mode == "sem-ge-imm":
                t = w.wait_value
                if 1 <= t <= K:
                    w.wait_value = next_kept_at[t]

    # drop updates from non-kept matmuls
    for inst, keep in clocked:
        if not keep:
            si = inst.sync_info
            si.on_update = [u for u in si.on_update
                            if not (u.id == pe_sem
                                    and u.update_mode == "sem-inc")]
    return nc


def _build_program(s_deq):
    """Build the per-core Bass program (identical across cores; data-parallel).

    s_deq[l] = in_scale[l]*w_scale[l] as python floats (fp32-exact values)
    """
    import concourse.bass as bass
    import concourse.mybir as mybir
    import concourse.tile as tile

    _install_drain_patch()
    dt = mybir.dt
    Alu = mybir.AluOpType
    Act = mybir.ActivationFunctionType

    nc = bass.Bass()
    W_d = nc.dram_tensor("wt", [NUM_LAYERS, KT, P, KT, P], dt.bfloat16,
                         kind="ExternalInput")
    X_d = nc.dram_tensor("xq0", [KT, P, NLOC], dt.bfloat16, kind="ExternalInput")
    G_d = nc.dram_tensor("gam", [NUM_LAYERS - 1, P, KT], dt.float32,
                         kind="ExternalInput")
    B_d = nc.dram_tensor("bet", [NUM_LAYERS - 1, P, KT], dt.float32,
                         kind="ExternalInput")
    O_d = nc.dram_tensor("out", [D, NLOC], dt.float32, kind="ExternalOutput")

    f32, f32r, bf16 = dt.float32, dt.float32r, dt.bfloat16

    with tile.TileContext(nc) as tc:
        with (
            tc.tile_pool(name="xq", bufs=64) as xq_pool,
            tc.tile_pool(name="h", bufs=33) as h_pool,
            tc.tile_pool(name="w", bufs=4) as w_pool,
            tc.tile_pool(name="sq", bufs=2) as sq_pool,
            tc.tile_pool(name="acc", bufs=4) as acc_pool,
            tc.tile_pool(name="st", bufs=6) as st_pool,
            tc.tile_pool(name="hilo", bufs=2) as hilo_pool,
            tc.tile_pool(name="wc", bufs=8) as wc_pool,
            tc.tile_pool(name="gb", bufs=6) as gb_pool,
            tc.tile_pool(name="const", bufs=1) as const_pool,
            tc.tile_pool(name="mmps", bufs=4, space="PSUM") as mm_ps,
            tc.tile_pool(name="stps", bufs=2, space="PSUM") as st_ps,
            tc.tile_pool(name="bcps", bufs=2, space="PSUM") as bc_ps,
        ):
            ones_f = const_pool.tile([P, 1], f32)
            nc.vector.memset(ones_f[:], 1.0)
            ones = const_pool.tile([P, 1], f32r)
            nc.vector.tensor_copy(ones[:], ones_f[:])
            eps = const_pool.tile([1, 1], f32)
            nc.vector.memset(eps[:], 1e-5)
            ones_row_f = const_pool.tile([1, P], f32)
            nc.vector.memset(ones_row_f[:], 1.0)

            # critical-path weight prefetch. The first two tiles arrive as
            # 8-ktile chunks so the very first matmuls only wait ~1us for
            # 256KB instead of 2.9us for the full 1MB tile.
            dma_engines = [nc.sync, nc.scalar, nc.gpsimd]
            rr = [0]

            def dma_rr(dst, src):
                dma_engines[rr[0] % 3].dma_start(dst, src)
                rr[0] += 1

            pre_w, pre_wc = {}, {}
            for c in range(4):
                t = wc_pool.tile([P, 8, P], bf16, tag="wc")
                dma_rr(t[:], W_d[0, 0, :, c * 8:(c + 1) * 8, :])
                pre_wc.setdefault(0, []).append(t)

            # half-0 activations are on the first matmuls' critical path;
            # half-1 descriptors are issued later, inside phase (0,0).
            # descriptor issue serializes at ~600ns per engine queue, so
            # fan the critical ones out across three engines
            xq_tiles = {}
            for kt in range(KT):
                t = xq_pool.tile([P, HALF], bf16, tag="xq")
                dma_rr(t[:], X_d[kt, :, 0:HALF])
                xq_tiles[(0, 0, kt)] = t
            for c in range(4):
                t = wc_pool.tile([P, 8, P], bf16, tag="wc")
                dma_rr(t[:], W_d[0, 1, :, c * 8:(c + 1) * 8, :])
                pre_wc.setdefault(1, []).append(t)
            for ot in (2, 3):
                t = w_pool.tile([P, KT, P], bf16, tag="w")
                dma_rr(t[:], W_d[0, ot])
                pre_w[ot] = t

            # gamma' = gamma/in_scale[l+1], beta' = beta/in_scale[l+1],
            # one strided DMA per layer each: [P, KT] tiles
            G1, B1 = [], []
            for l in range(NUM_LAYERS - 1):
                g = gb_pool.tile([P, KT], f32, tag="gb")
                nc.sync.dma_start(g[:], G_d[l])
                G1.append(g)
                b = gb_pool.tile([P, KT], f32, tag="gb")
                nc.sync.dma_start(b[:], B_d[l])
                B1.append(b)

            h_tiles = {}

            def emit_stats(pv):
                """Partition-reduce the accumulated sums; tiny row math.
                f32r truncates the moving operand to ~bf16, so send each
                accumulator as a rounded part plus residual (exact in sum)."""
                l, half, accS, accQ = pv["l"], pv["half"], pv["accS"], pv["accQ"]
                S_ps = st_ps.tile([1, HALF], f32, tag="stps")
                Q_ps = st_ps.tile([1, HALF], f32, tag="stps")
                for acc, ps in ((accS, S_ps), (accQ, Q_ps)):
                    hi = hilo_pool.tile([P, HALF], f32r, tag="hilo")
                    nc.vector.tensor_copy(hi[:], acc[:])
                    lo = hilo_pool.tile([P, HALF], f32r, tag="hilo")
                    nc.vector.tensor_tensor(lo[:], acc[:], hi[:].bitcast(f32),
                                            op=Alu.subtract)
                    nc.tensor.matmul(ps[:], ones[:], hi[:],
                                     start=True, stop=False,
                                     skip_group_check=True)
                    nc.tensor.matmul(ps[:], ones[:], lo[:],
                                     start=False, stop=True,
                                     skip_group_check=True)
                mu = st_pool.tile([1, HALF], f32, tag="st")
                nc.vector.tensor_scalar_mul(mu[:], S_ps[:], 1.0 / D)
                q = st_pool.tile([1, HALF], f32, tag="st")
                nc.vector.tensor_scalar_mul(q[:], Q_ps[:], 1.0 / D)
                var = st_pool.tile([1, HALF], f32, tag="st")
                nc.vector.tensor_tensor(var[:], mu[:], mu[:], op=Alu.mult)
                nc.vector.tensor_tensor(var[:], q[:], var[:], op=Alu.subtract)
                std = st_pool.tile([1, HALF], f32, tag="st")
                nc.scalar.activation(std[:], var[:], Act.Sqrt, bias=eps[:])
                rstd = st_pool.tile([1, HALF], f32, tag="st")
                nc.vector.reciprocal(rstd[:], std[:])
                nmr = st_pool.tile([1, HALF], f32, tag="st")
                nc.vector.scalar_tensor_tensor(
                    nmr[:], mu[:], -1.0, rstd[:], op0=Alu.mult, op1=Alu.mult)
                pv["rstd"], pv["nmr"] = rstd, nmr

            def emit_bcast(pv):
                """Broadcast the per-token rows across partitions.
                fp32 x fp32 broadcast matmuls are slower (multi-pass) but
                bit-exact; f32r here truncates rstd and shifts quant steps."""
                rstdB = bc_ps.tile([P, HALF], f32, tag="bcps")
                nc.tensor.matmul(rstdB[:], ones_row_f[:], pv["rstd"][:],
                                 start=True, stop=True, skip_group_check=True)
                muB = bc_ps.tile([P, HALF], f32, tag="bcps")
                nc.tensor.matmul(muB[:], ones_row_f[:], pv["nmr"][:],
                                 start=True, stop=True, skip_group_check=True)
                pv["rstdB"], pv["muB"] = rstdB, muB

            def emit_post_chunk(pv, ft0, ft1):
                """Normalize + requantize h tiles [ft0, ft1) into next
                layer's xq. gamma'/beta'' multiply-add rides the idle ACT
                engine; beta'' carries +MAGIC so the fp32 write rounds to
                integer (RNE) in the same op."""
                l, half = pv["l"], pv["half"]
                rstdB, muB = pv["rstdB"], pv["muB"]
                for ft in range(ft0, ft1):
                    h_t = h_tiles.pop((half, ft))
                    nc.vector.tensor_tensor(h_t[:], h_t[:], rstdB[:],
                                            op=Alu.mult)
                    nc.vector.tensor_tensor(h_t[:], h_t[:], muB[:],
                                            op=Alu.add)
                    nc.scalar.activation(h_t[:], h_t[:], Act.Identity,
                                         scale=G1[l][:, ft:ft + 1],
                                         bias=B1[l][:, ft:ft + 1])
                    xq_t = xq_pool.tile([P, HALF], bf16, tag="xq")
                    nc.vector.tensor_scalar(xq_t[:], h_t[:], MAGIC + 127.0,
                                            MAGIC, op0=Alu.min,
                                            op1=Alu.subtract)
                    nc.vector.tensor_scalar_max(xq_t[:], xq_t[:], -128.0)
                    xq_tiles[(l + 1, half, ft)] = xq_t

            def emit_phase(l, half, prev):
                last = l == NUM_LAYERS - 1
                if not last:
                    accS = acc_pool.tile([P, HALF], f32, tag="acc")
                    accQ = acc_pool.tile([P, HALF], f32, tag="acc")
                for ot in range(KT):
                    wsrc = None
                    if l == 0 and half == 0 and ot in pre_wc:
                        ch = pre_wc[ot]
                        wsrc = lambda kt: ch[kt // 8][:, kt % 8, :]
                    elif l == 0 and half == 0 and ot in pre_w:
                        w = pre_w[ot]
                    else:
                        w = w_pool.tile([P, KT, P], bf16, tag="w")
                        nc.sync.dma_start(w[:], W_d[l, ot])
                    if wsrc is None:
                        wsrc = lambda kt: w[:, kt, :]
                    ps = mm_ps.tile([P, HALF], f32, tag="mmps")
                    for kt in range(KT):
                        nc.tensor.matmul(
                            ps[:], wsrc(kt), xq_tiles[(l, half, kt)][:],
                            start=(kt == 0), stop=(kt == KT - 1),
                            skip_group_check=True)
                    if not last:
                        h_t = h_pool.tile([P, HALF], f32, tag="h")
                        nc.scalar.activation(h_t[:], ps[:], Act.Relu,
                                             scale=float(s_deq[l]))
                        sq = sq_pool.tile([P, HALF], f32, tag="sq")
                        nc.vector.tensor_tensor(sq[:], h_t[:], h_t[:],
                                                op=Alu.mult)
                        if ot == 0:
                            nc.vector.tensor_copy(accS[:], h_t[:])
                            nc.vector.tensor_copy(accQ[:], sq[:])
                        else:
                            nc.vector.tensor_tensor(
                                accS[:], accS[:], h_t[:], op=Alu.add)
                            nc.vector.tensor_tensor(
                                accQ[:], accQ[:], sq[:], op=Alu.add)
                        h_tiles[(half, ot)] = h_t
                    else:
                        h_t = h_pool.tile([P, HALF], f32, tag="h")
                        nc.scalar.activation(h_t[:], ps[:], Act.Copy,
                                             scale=float(s_deq[l]))
                        nc.sync.dma_start(
                            O_d[ot * P:(ot + 1) * P,
                                half * HALF:(half + 1) * HALF], h_t[:])
                    if prev is not None:
                        if ot == 1:
                            emit_stats(prev)
                        elif ot == 3:
                            emit_bcast(prev)
                        elif 4 <= ot <= 25 and (ot - 4) % 3 == 0:
                            ck = (ot - 4) // 3
                            emit_post_chunk(prev, ck * 4, ck * 4 + 4)
                    if l == 0 and half == 0 and ot == 8:
                        for kt in range(KT):
                            t = xq_pool.tile([P, HALF], bf16, tag="xq")
                            nc.sync.dma_start(t[:], X_d[kt, :, HALF:NLOC])
                            xq_tiles[(0, 1, kt)] = t
                if not last:
                    return {"l": l, "half": half, "accS": accS, "accQ": accQ}
                return None

            prev = None
            for l in range(NUM_LAYERS):
                for half in range(2):
                    prev = emit_phase(l, half, prev)

    _trim_pe_sem_updates(nc)
    return _split_excess_waits(nc)


def kernel(x, Ws, w_scales, in_scales, gammas, betas, _trace=False):
    import ml_dtypes
    from concourse.bass_utils import run_bass_kernel_spmd

    f32 = np.float32
    C = f32(MAGIC)
    x = np.asarray(x, f32)
    Ws = np.asarray(Ws, f32)
    w_scales = np.asarray(w_scales, f32)
    in_scales = np.asarray(in_scales, f32)
    gammas = np.asarray(gammas, f32)
    betas = np.asarray(betas, f32)

    # ---- host prep (offline-weight-style preprocessing) ----
    # ternary quantize weights; XLA divides by reciprocal-multiply and
    # rounds nearest-even, both reproduced here bitwise.
    WT = np.empty((NUM_LAYERS, KT, P, KT, P), ml_dtypes.bfloat16)
    for l in range(NUM_LAYERS):
        wq = ((Ws[l] * (f32(1.0) / w_scales[l])) + C) - C
        wq = np.clip(wq, -1.0, 1.0).astype(f32)
        # WT[l, ot, kp, kt, o] = wq[ot*128+o, kt*128+kp]
        t = wq.reshape(KT, P, KT, P)          # [ot, o, kt, kp]
        WT[l] = t.transpose(0, 3, 2, 1).astype(ml_dtypes.bfloat16)

    xq0 = ((x * (f32(1.0) / in_scales[0])) + C) - C
    xq0 = np.clip(xq0, -128.0, 127.0).astype(f32)
    xT = np.ascontiguousarray(xq0.T)           # [k, n]

    # gamma' = gamma/in_scale[l+1], beta' = beta/in_scale[l+1],
    # laid out [layer, partition, feature-tile] for single-DMA loads
    inv_in = [f32(1.0) / in_scales[l] for l in range(NUM_LAYERS)]
    G = np.empty((NUM_LAYERS - 1, P, KT), f32)
    B = np.empty((NUM_LAYERS - 1, P, KT), f32)
    for l in range(NUM_LAYERS - 1):
        G[l] = (gammas[l] * inv_in[l + 1]).reshape(KT, P).T
        # +MAGIC rides the beta bias so the ACT fp32 write rounds to int
        B[l] = (betas[l] * inv_in[l + 1] + C).reshape(KT, P).T

    s_deq = [float(in_scales[l] * w_scales[l]) for l in range(NUM_LAYERS)]

    key = tuple(s_deq)
    if key not in _prog_cache:
        _prog_cache[key] = _build_program(s_deq)
    nc = _prog_cache[key]

    in_maps = []
    for c in range(NUM_CORES):
        xs = xT[:, c * NLOC:(c + 1) * NLOC].reshape(KT, P, NLOC)
        in_maps.append({
            "wt": WT,
            "xq0": np.ascontiguousarray(xs).astype(ml_dtypes.bfloat16),
            "gam": G,
            "bet": B,
        })

    res = run_bass_kernel_spmd(nc, in_maps, list(range(NUM_CORES)),
                               trace=_trace)
    if _trace:
        kernel.last_exec_ns = res.exec_time_ns

    outT = np.concatenate(
        [res.results[c]["out"] for c in range(NUM_CORES)], axis=1)
    return np.ascontiguousarray(outT.T).astype(np.float32)


kernel.last_exec_ns = None
